# revision 30
# baseline (speedup 1.0000x reference)
"""Trainium2 Bass kernel for fused attention block (QKV+gate proj, q/k RMS-norm,
RoPE, causal GQA attention, sigmoid gating, o_proj).

Sharding: 8 cores = 2 batches x 4 head-groups (tensor-parallel over heads,
data-parallel over batch). Each core computes a partial [T, D] output from its
4 q-heads / 1 kv-head; host sums the 4 partials per batch.

Self-contained: hardcodes all shapes; reads nothing from /root/problem.
"""

import numpy as np
import ml_dtypes

import concourse.bass as bass
import concourse.bacc as bacc
import concourse.mybir as mybir
import concourse.tile as tile
from concourse.bass import ts, ds
from concourse.bass_utils import run_bass_kernel_spmd

# ---- problem constants ----
B, T, D = 2, 2048, 2048
NH, NKV, HD = 16, 4, 128
NQ = NH // NKV          # q heads per core
DH = NQ * HD            # 512 (attn feature rows per core)
EPS = 1e-6
SCALE = HD ** -0.5
TB = 512                # moving free-dim block
NTB = T // TB           # 4
NKT = D // 128          # 16 contraction tiles
NTT = T // 128          # 16 t(row)-tiles

F32 = mybir.dt.float32
BF16 = mybir.dt.bfloat16
AF = mybir.ActivationFunctionType
ALU = mybir.AluOpType
NPBF16 = ml_dtypes.bfloat16

MMDT = BF16


def _mm(nc, out, lhsT, rhs, **kw):
    nc.tensor.matmul(out, lhsT, rhs, **kw)


def _emit(tc, io):
    nc = tc.nc
    with (
        tc.tile_pool(name="consts", bufs=1) as cpool,
        tc.tile_pool(name="persist", bufs=1) as ppool,
        tc.tile_pool(name="xt", bufs=2) as xpool,
        tc.tile_pool(name="workB", bufs=2) as wb,
        tc.tile_pool(name="probs", bufs=6) as prp,
        tc.tile_pool(name="workC", bufs=2) as wc,
        tc.tile_pool(name="outp", bufs=4) as op,
        tc.tile_pool(name="ps_acc", bufs=4, space="PSUM") as ps_acc,
        tc.tile_pool(name="ps_sc", bufs=4, space="PSUM") as ps_sc,
    ):
        # ---------- DMA order: first-use order, split across the two HWDGE
        # queues (sync / scalar) so early tiles land just in time ----------
        xT_r = io["xT"].rearrange("(ko p) t -> p ko t", p=128)
        xts = {}
        xts[0] = [xpool.tile([128, 4, TB], MMDT, name=f"xt0_{ck}", tag=f"xt{ck}")
                  for ck in range(4)]
        wq_sbs = [cpool.tile([128, NKT, HD], MMDT, name=f"wq_sb{h}")
                  for h in range(NQ)]
        # sync queue: xt chunks interleaved with per-head q weights (first use)
        for ck in range(4):
            nc.sync.dma_start(xts[0][ck][:], xT_r[:, ts(ck, 4), ds(0, TB)])
            nc.sync.dma_start(wq_sbs[ck][:], io["wq"][:, ck, :, :])
        wv_sb = cpool.tile([128, NKT, HD], MMDT, name="wv_sb")
        nc.sync.dma_start(wv_sb[:], io["wv"][:, :, :])
        tri_sb = cpool.tile([128, 128], MMDT, name="tri_sb")
        nc.sync.dma_start(tri_sb[:], io["tri"][:, :])
        # scalar queue: k weights, rope tables, gate weights, o weights
        wk_sb = cpool.tile([128, NKT, HD], MMDT, name="wk_sb")
        nc.scalar.dma_start(wk_sb[:], io["wk"][:, :, :])
        cosk_sb = cpool.tile([128, T], MMDT, name="cosk_sb")
        nc.scalar.dma_start(cosk_sb[:], io["coskT"][:, :])
        sink_sb = cpool.tile([128, T], MMDT, name="sink_sb")
        nc.scalar.dma_start(sink_sb[:], io["sinkT"][:, :])
        cosq_sb = cpool.tile([128, T], MMDT, name="cosq_sb")
        nc.scalar.dma_start(cosq_sb[:], io["cosqT"][:, :])
        sinq_sb = cpool.tile([128, T], MMDT, name="sinq_sb")
        nc.scalar.dma_start(sinq_sb[:], io["sinqT"][:, :])
        wg_sbs = [cpool.tile([128, NKT, HD], MMDT, name=f"wg_sb{h}")
                  for h in range(NQ)]
        for h in range(NQ):
            nc.scalar.dma_start(wg_sbs[h][:], io["wg"][:, h, :, :])
        wo_sb = cpool.tile([128, NQ, D], MMDT, name="wo_sb")
        wo_r = io["wo"].rearrange("(h p) n -> p h n", p=128)
        for h in range(NQ):
            nc.scalar.dma_start(wo_sb[:, h, :], wo_r[:, h, :])
        ones_sb = cpool.tile([128, 128], MMDT, name="ones_sb")
        nc.gpsimd.memset(ones_sb[:], 1.0)
        eps_sb = cpool.tile([128, 1], F32, name="eps_sb")
        nc.gpsimd.memset(eps_sb[:], EPS)
        ident_sb = cpool.tile([128, 128], MMDT, name="ident_sb")
        from concourse.masks import make_identity
        make_identity(nc, ident_sb[:])

        # ---------- persistent activations ----------
        qrope = ppool.tile([128, NQ, T], MMDT, name="qrope")
        krope = ppool.tile([128, T], MMDT, name="krope")
        sg = ppool.tile([128, NQ, T], MMDT, name="sg")       # exp(-gate)
        v_sb = ppool.tile([128, NTT, HD], MMDT, name="v_sb")
        attnT_t = [ppool.tile([128, NQ, TB], MMDT, name=f"attnT{i}")
                   for i in range(NTB)]

        def emit_oproj_group(src_tb, g):
            """4 of the 16 o_proj tiles for query-block src_tb."""
            for idx in range(4 * g, 4 * g + 4):
                tt, nb = divmod(idx, 4)
                ti = src_tb * 4 + tt
                pso = ps_acc.tile([128, TB], F32, name=f"pso_{ti}_{nb}", tag="acc")
                for h in range(NQ):
                    _mm(nc, pso, attnT_t[src_tb][:, h, ts(tt, 128)],
                        wo_sb[:, h, ts(nb, TB)],
                        start=(h == 0), stop=(h == NQ - 1))
                osb = op.tile([128, TB], MMDT, name=f"osb_{ti}_{nb}", tag="osb")
                nc.vector.tensor_copy(osb[:], pso[:])
                nc.sync.dma_start(io["out"][ts(ti, 128), ts(nb, TB)], osb[:])

        for tb in range(NTB):
            tsl = ds(tb * TB, TB)
            xt = xts.pop(tb)      # list of 4 chunk tiles

            # ======== Phase B: QKV projection + norm + rope ========
            qk_specs = [("k", 0)] + [("q", h) for h in range(NQ)]
            tails = {}
            vt_store = {}

            def accum_qk(i, tb=tb, xt=xt, tails=tails, qk_specs=qk_specs):
                kind, h = qk_specs[i]
                w_sb = wq_sbs[h] if kind == "q" else wk_sb
                ps = ps_acc.tile([128, TB], F32, name=f"psqk_{tb}_{i}", tag="acc")
                for kt in range(NKT):
                    ck, kk = divmod(kt, 4)
                    _mm(nc, ps, w_sb[:, kt, :], xt[ck][:, kk, :],
                        start=(kt == 0), stop=(kt == NKT - 1))
                qsb = wb.tile([128, TB], MMDT, name=f"qsb_{tb}_{i}", tag="qsb")
                nc.vector.tensor_copy(qsb[:], ps[:])
                sq = wb.tile([128, TB], MMDT, name=f"sq_{tb}_{i}", tag="sq")
                nc.vector.tensor_mul(sq[:], qsb[:], qsb[:])
                tails[i] = {"sq": sq, "qsb": qsb, "kind": kind, "h": h}

            def tail_var(i, tb=tb, tails=tails):
                """ones128 @ sq = variance already broadcast to all partitions;
                then rstd = 1/sqrt(v/HD + eps) via ACT Sqrt + DVE fast recip."""
                st = tails[i]
                vps = ps_sc.tile([128, TB], F32, name=f"var_{tb}_{i}", tag="sc")
                _mm(nc, vps, ones_sb[:, :], st["sq"][:, :], start=True, stop=True)
                srep = wb.tile([128, TB], F32, name=f"srep_{tb}_{i}", tag="srep")
                nc.scalar.activation(srep[:], vps[:], AF.Sqrt,
                                     bias=eps_sb[:, 0:1], scale=1.0 / HD)
                vrep = wb.tile([128, TB], F32, name=f"vrep_{tb}_{i}", tag="vrep")
                nc.vector.reciprocal_approx_fast(out=vrep[:], in_=srep[:])
                st["vrep"] = vrep

            def tail_rope(i, tb=tb, tsl=tsl, tails=tails):
                st = tails[i]
                qsb = st["qsb"]
                rot = wb.tile([128, TB], MMDT, name=f"rot_{tb}_{i}", tag="rot")
                nc.vector.tensor_scalar_mul(rot[0:64, :], qsb[64:128, :], -1.0)
                nc.vector.tensor_copy(rot[64:128, :], qsb[0:64, :])
                cos_sb = cosq_sb if st["kind"] == "q" else cosk_sb
                sin_sb = sinq_sb if st["kind"] == "q" else sink_sb
                t1 = wb.tile([128, TB], MMDT, name=f"t1_{tb}_{i}", tag="t1")
                nc.vector.tensor_mul(t1[:], qsb[:], cos_sb[:, tsl])
                t2 = wb.tile([128, TB], MMDT, name=f"t2_{tb}_{i}", tag="t2")
                nc.vector.tensor_mul(t2[:], rot[:], sin_sb[:, tsl])
                t3 = wb.tile([128, TB], MMDT, name=f"t3_{tb}_{i}", tag="t3")
                nc.vector.tensor_add(t3[:], t1[:], t2[:])
                dst = qrope[:, st["h"], tsl] if st["kind"] == "q" else krope[:, tsl]
                nc.vector.tensor_mul(dst, t3[:], st["vrep"][:])

            def accum_gate(h, tb=tb, xt=xt, tsl=tsl):
                ps = ps_acc.tile([128, TB], F32, name=f"psg_{tb}_{h}", tag="acc")
                for kt in range(NKT):
                    ck, kk = divmod(kt, 4)
                    _mm(nc, ps, wg_sbs[h][:, kt, :], xt[ck][:, kk, :],
                        start=(kt == 0), stop=(kt == NKT - 1))
                # store exp(-g); sigmoid(g) = 1/(1+exp(-g)) applied later
                nc.scalar.activation(sg[:, h, tsl], ps[:], AF.Exp, scale=-1.0)

            def accum_vT(tb=tb, xt=xt):
                ps = ps_acc.tile([128, TB], F32, name=f"psvT_{tb}", tag="acc")
                for kt in range(NKT):
                    ck, kk = divmod(kt, 4)
                    _mm(nc, ps, wv_sb[:, kt, :], xt[ck][:, kk, :],
                        start=(kt == 0), stop=(kt == NKT - 1))
                vt = wb.tile([128, TB], MMDT, name=f"vt_{tb}", tag="vt")
                nc.vector.tensor_copy(vt[:], ps[:])
                vt_store[tb] = vt

            def transpose_v(tt, tb=tb):
                ti = tb * 4 + tt
                ps = ps_sc.tile([128, HD], MMDT, name=f"psvt_{tb}_{tt}", tag="sc")
                nc.tensor.transpose(ps[:], vt_store[tb][:, ts(tt, 128)], ident_sb[:])
                nc.vector.tensor_copy(v_sb[:, ti, :], ps[:])

            blocks = [lambda i=i: accum_qk(i) for i in range(5)] + [accum_vT]
            tail_sched = {}
            for i in range(5):
                tail_sched.setdefault(i + 1, []).append(lambda i=i: tail_var(i))
                if i + 2 <= 5:
                    tail_sched.setdefault(i + 2, []).append(lambda i=i: tail_rope(i))
            for bi, blk in enumerate(blocks):
                blk()
                for fn in tail_sched.get(bi + 1, ()):
                    fn()
            tail_rope(4)
            for tt in range(4):
                transpose_v(tt)

            # ======== Phase C: attention ========
            # prefetch next block's x tile while PE chews on attention
            if tb + 1 < NTB:
                xt_next = [xpool.tile([128, 4, TB], MMDT, name=f"xt{tb+1}_{ck}",
                                      tag=f"xt{ck}") for ck in range(4)]
                for ck in range(4):
                    nc.sync.dma_start(xt_next[ck][:],
                                      xT_r[:, ts(ck, 4), ds((tb + 1) * TB, TB)])
                xts[tb + 1] = xt_next

            nj = 4 * (tb + 1)
            for h in range(NQ):
                attn_ps = ps_acc.tile([128, TB], F32, name=f"attn_{tb}_{h}", tag="acc")
                # ones128 @ pr accumulates the softmax denominator already
                # broadcast across all 128 partitions
                den_ps = ps_acc.tile([128, TB], F32, name=f"den_{tb}_{h}", tag="acc")
                probs_t = [None] * nj

                def emit_scores(j, h=h, tb=tb, probs_t=probs_t):
                    o = j - 4 * tb
                    c0 = max(0, o) * 128      # first valid column in this tile
                    w = TB - c0
                    sp = ps_sc.tile([128, TB], F32, name=f"sc_{tb}_{h}_{j}", tag="sc")
                    _mm(nc, sp[:, c0:], krope[:, ts(j, 128)],
                        qrope[:, h, ds(tb * TB + c0, w)], start=True, stop=True)
                    pr = prp.tile([128, TB], MMDT, name=f"pr_{tb}_{h}_{j}", tag="pr")
                    nc.scalar.activation(pr[:, c0:], sp[:, c0:], AF.Exp, scale=SCALE)
                    if o >= 0:
                        nc.vector.tensor_mul(pr[:, c0:c0 + 128], pr[:, c0:c0 + 128],
                                             tri_sb[:, :])
                    probs_t[j] = pr

                def emit_av(j, h=h, tb=tb, nj=nj, probs_t=probs_t, den_ps=den_ps,
                            attn_ps=attn_ps):
                    o = j - 4 * tb
                    c0 = max(0, o) * 128
                    pr = probs_t[j]
                    _mm(nc, attn_ps[:, c0:], v_sb[:, j, :], pr[:, c0:],
                        start=(j == 0), stop=(j == nj - 1))
                    _mm(nc, den_ps[:, c0:], ones_sb[:, :], pr[:, c0:],
                        start=(j == 0), stop=(j == nj - 1))

                LOOK = 2
                for j in range(nj):
                    emit_scores(j)
                    if j >= LOOK:
                        emit_av(j - LOOK)
                for j in range(max(0, nj - LOOK), nj):
                    emit_av(j)

                # gate projection deferred here: removes wg from the
                # bandwidth-bound early window, fills head-boundary bubbles
                accum_gate(h)

                # normalize + gate: attnT = attn / ((1 + exp(-g)) * den)
                u = wc.tile([128, TB], F32, name=f"u_{tb}_{h}", tag="u")
                nc.vector.scalar_tensor_tensor(
                    u[:], sg[:, h, tsl], 1.0, den_ps[:], op0=ALU.add, op1=ALU.mult)
                r = wc.tile([128, TB], F32, name=f"r_{tb}_{h}", tag="r")
                nc.vector.reciprocal_approx_fast(out=r[:], in_=u[:])
                nc.vector.tensor_mul(attnT_t[tb][:, h, :], attn_ps[:], r[:])

                # deferred o_proj of previous block fills PE bubbles here
                if tb > 0:
                    emit_oproj_group(tb - 1, h)

        # final o_proj for the last query block
        for g in range(4):
            emit_oproj_group(NTB - 1, g)


_CACHED = {}


def _build():
    if "nc" in _CACHED:
        return _CACHED["nc"]
    nc = bacc.Bacc("TRN2", target_bir_lowering=False, debug=False, num_devices=8)
    io = {}
    def din(name, shape, dt):
        io[name] = nc.dram_tensor(name, shape, dt, kind="ExternalInput").ap()
    din("xT", [D, T], MMDT)
    din("wq", [128, NQ, NKT, HD], MMDT)
    din("wg", [128, NQ, NKT, HD], MMDT)
    din("wk", [128, NKT, HD], MMDT)
    din("wv", [128, NKT, HD], MMDT)
    din("wo", [DH, D], MMDT)
    din("cosqT", [HD, T], MMDT)
    din("sinqT", [HD, T], MMDT)
    din("coskT", [HD, T], MMDT)
    din("sinkT", [HD, T], MMDT)
    din("tri", [128, 128], MMDT)
    io["out"] = nc.dram_tensor("out", [T, D], MMDT, kind="ExternalOutput").ap()

    with tile.TileContext(nc, num_cores=8) as tc:
        _emit(tc, io)
    nc.compile()
    _CACHED["nc"] = nc
    return nc


def _prep_in_maps(inputs):
    hidden = np.asarray(inputs["hidden_BTD"], np.float32)
    cos = np.asarray(inputs["cos_BTK"], np.float32)
    sin = np.asarray(inputs["sin_BTK"], np.float32)
    w_q = np.asarray(inputs["w_q"], np.float32)
    w_k = np.asarray(inputs["w_k"], np.float32)
    w_v = np.asarray(inputs["w_v"], np.float32)
    w_o = np.asarray(inputs["w_o"], np.float32)
    qw = np.asarray(inputs["q_norm_w"], np.float32)
    kw = np.asarray(inputs["k_norm_w"], np.float32)

    wq4 = w_q.reshape(D, NH, 2 * HD)

    def cvt(x):
        return np.ascontiguousarray(np.asarray(x, np.float32).astype(NPBF16))

    def pack_w(w):
        # [D, HD] -> [128, NKT, HD] tile layout (contiguous per partition)
        return w.reshape(NKT, 128, HD).transpose(1, 0, 2)

    # upper-tri-inclusive [128,128]: tri[jl, cc] = 1 iff jl <= cc
    tri = np.triu(np.ones((128, 128), np.float32))

    # rms-norm weights folded into the rope tables (exact: per-partition
    # factors commute with rotate-half using the permuted index)
    perm_idx = (np.arange(HD) + 64) % HD

    in_maps = []
    for c in range(8):
        b, g = divmod(c, 4)
        heads = list(range(4 * g, 4 * g + 4))
        cosT = cos[b].T          # [HD, T]
        sinT = sin[b].T
        m = {
            "xT": cvt(hidden[b].T),
            "wq": cvt(np.stack([pack_w(wq4[:, h, :HD]) for h in heads])
                      .transpose(1, 0, 2, 3)),
            "wg": cvt(np.stack([pack_w(wq4[:, h, HD:]) for h in heads])
                      .transpose(1, 0, 2, 3)),
            "wk": cvt(pack_w(w_k[:, g * HD:(g + 1) * HD])),
            "wv": cvt(pack_w(w_v[:, g * HD:(g + 1) * HD])),
            "wo": cvt(w_o[4 * g * HD:(4 * g + 4) * HD, :]),
            "cosqT": cvt(cosT * qw[:, None]),
            "sinqT": cvt(sinT * qw[perm_idx, None]),
            "coskT": cvt(cosT * kw[:, None]),
            "sinkT": cvt(sinT * kw[perm_idx, None]),
            "tri": cvt(tri),
        }
        in_maps.append(m)
    return in_maps


def run(inputs, **spmd_kwargs):
    """Build+run; returns (full_output [B,T,D] fp32, BassKernelResults)."""
    nc = _build()
    in_maps = _prep_in_maps(inputs)
    res = run_bass_kernel_spmd(nc, in_maps, core_ids=list(range(8)), **spmd_kwargs)
    out = np.zeros((B, T, D), np.float32)
    for c in range(8):
        out[c // 4] += np.asarray(res.results[c]["out"], np.float32)
    return out, res


def kernel(**inputs):
    out, _ = run(inputs)
    return out
